# revision 1
# baseline (speedup 1.0000x reference)
"""CompoundLoss (dice + focal + edge) Trainium2 Bass kernel.

Self-contained: hardcodes shapes [8,11,512,512] f32 logits + [8,512,512] i32
targets, shards batch across 8 NeuronCores (pure data parallel). Each core
reduces its image to 94 per-partition fp32 accumulator columns; the host
finishes the tiny scalar math in fp64.

Math notes (per image):
  softmax: E_c = exp(L_c) (bf16), Z = sum_c E_c (PE identity-matmul accumulate
  into PSUM, fp32), r = exp(-ln Z) via ACT. pt = (sum_c [T==c]*E_c) * r.
  dice: inter[c] = sum_p [T==c]*pt, card[c] = sum_p E_c*r + count[c] + eps.
  focal: mean(-0.25*(1-pt)^2*ln(pt)).
  edge: multi-hot preds pm_c = [E_c >= max_c E_c] packed as PB = sum 2^c*pm_c
  (PE scaled-identity accumulate). Targets packed as one-hot bitmask
  bm = 1<<T (int16). 3x3 OR/AND window convs + 4-neighbor OR via shifted-AP
  TTs and partition-shift DMAs give bmOR (class presence in window), bmAND
  (uniform window), bm4 (4-neighbor presence). Then per class c=1..10:
    num[c] = y1 - y2 + es2*y3 + (e1-es2)*y4,  den[c] = denP - denN
    y1 = #bit_c(bm & ~PB)    (single-bit plane -> is_equal-2^c + fused accum)
    y2 = #bit_c(bmAND & ~PB) (single-bit)
    y3 = #bit_c(PB & ~bm & bmOR), y4 = #bit_c(PB & ~bm & bm4)  (2-op extract)
    denP = #bit_c(bmOR) (2-op), denN = #bit_c(bmAND) (single-bit)
"""

import numpy as np

B, C, H, W = 8, 11, 512, 512
P = 128
KB = H // P          # 4 row-blocks
NF = KB * W          # 2048 free elems per partition
NPIX = H * W
EPS = 1e-6
E1 = float(np.exp(-1.0))
ES2 = float(np.exp(-np.sqrt(2.0)))

# stats column layout
COL_INTER = 0          # 11
COL_SUMP = 11          # 11
COL_COUNT = 22         # 11
COL_FOCAL = 33         # 1
COL_Y1 = 34            # 10 (c=1..10)
COL_Y2 = 44
COL_Y3 = 54
COL_Y4 = 64
COL_DENP = 74
COL_DENN = 84
NCOL = 94

_cache = {}


def _build():
    import ml_dtypes
    import concourse.bacc as bacc
    import concourse.mybir as mybir
    from concourse.tile import TileContext

    f32 = mybir.dt.float32
    bf16 = mybir.dt.bfloat16
    i32 = mybir.dt.int32
    i16 = mybir.dt.int16
    op = mybir.AluOpType
    act = mybir.ActivationFunctionType

    nc = bacc.Bacc()
    x = nc.dram_tensor("x", [C, H, W], f32, kind="ExternalInput")
    t = nc.dram_tensor("t", [H, W], i32, kind="ExternalInput")
    stats_out = nc.dram_tensor("stats", [P, NCOL], f32, kind="ExternalOutput")
    statsa_out = nc.dram_tensor("statsa", [P, 32], f32, kind="ExternalOutput")
    statsp_out = nc.dram_tensor("statsp", [P, 4], f32, kind="ExternalOutput")

    # [C, 128, 4, 512] view: row = 128*k + p
    xv = x[:, :, :].rearrange("c (k p) w -> c p k w", p=P)
    tv = t[:, :].rearrange("(k p) w -> p k w", p=P)

    ident_np = np.eye(P, dtype=np.float32)
    ident_d = nc.inline_tensor(ident_np.astype(ml_dtypes.bfloat16), name="ident")
    # scaled identities 2^c for PB accumulation, c=1..10
    sid_np = np.stack([ident_np * float(1 << c) for c in range(1, 11)])
    sid_d = nc.inline_tensor(sid_np.astype(ml_dtypes.bfloat16), name="sident")
    # ones-column matrices: ocol_c[p, m] = [m == c]; ones-mm routes the
    # column-sum of rhs into PSUM partition-row c (zeros elsewhere)
    ocol_np = np.zeros((C, P, P), dtype=np.float32)
    for c_ in range(C):
        ocol_np[c_, :, c_] = 1.0
    ocol_d = nc.inline_tensor(ocol_np.astype(ml_dtypes.bfloat16), name="ocol")

    with TileContext(nc, pool_alloc_mode="queue") as tc:
        with (
            tc.tile_pool(name="persist", bufs=1) as pp,
            tc.tile_pool(name="scratch", bufs=2) as sp,
        ):
            # constants
            identb = pp.tile([P, P], bf16, name="identb")
            nc.sync.dma_start(identb, ident_d[:, :])
            sids = []
            for ci in range(10):
                sid = pp.tile([P, P], bf16, name=f"sid{ci}", tag=f"sid{ci}")
                nc.sync.dma_start(sid, sid_d[ci])
                sids.append(sid)
            ocols = []
            for ci in range(C):
                oc = pp.tile([P, P], bf16, name=f"oc{ci}", tag=f"oc{ci}")
                nc.sync.dma_start(oc, ocol_d[ci])
                ocols.append(oc)

            stats = pp.tile([P, NCOL], f32, name="stats")
            statsa = pp.tile([P, 32], f32, name="statsa")
            statsp = pp.tile([P, 4], f32, name="statsp")

            # targets
            ts32 = sp.tile([P, NF], i32, name="ts32", tag="f32s", bufs=1)
            nc.sync.dma_start(ts32.rearrange("p (k w) -> p k w", w=W), tv)
            t16b = pp.tile([P, NF], bf16, name="t16b")
            nc.vector.tensor_copy(t16b, ts32)
            t16i = pp.tile([P, NF], i16, name="t16i")
            nc.vector.tensor_copy(t16i, ts32)

            # ---- phase A: load logits, exp, Z accumulation in PSUM ----
            E = []
            _lcm = tc.tile_pool(name="lpool", bufs=2)
            _ecm = tc.tile_pool(name="epool", bufs=11)
            _mcm = tc.tile_pool(name="mxpool", bufs=4)
            _ccm2 = tc.tile_pool(name="cpool", bufs=2)
            lpool = _lcm.__enter__()
            epool = _ecm.__enter__()
            mxp = _mcm.__enter__()
            cp = _ccm2.__enter__()
            with tc.tile_pool(name="zpsum", bufs=1, space="PSUM") as zp:
                zps = zp.tile([P, NF], f32, name="zps")
                for c in range(C):
                    lb = lpool.tile([P, NF], f32, name=f"lb{c}", tag="lb")
                    nc.sync.dma_start(
                        lb.rearrange("p (k w) -> p k w", w=W), xv[c]
                    )
                    e = epool.tile([P, NF], bf16, name=f"e{c}", tag="e")
                    nc.scalar.activation(e, lb, act.Exp)
                    E.append(e)
                    for k in range(KB):
                        nc.tensor.matmul(
                            zps[:, k * W : (k + 1) * W],
                            identb,
                            e[:, k * W : (k + 1) * W],
                            start=(c == 0),
                            stop=(c == C - 1),
                        )
                # r = exp(-ln Z)
                lnz = sp.tile([P, NF], f32, name="lnz", tag="f32s", bufs=1)
                nc.scalar.activation(lnz, zps, act.Ln)
            r = pp.tile([P, NF], bf16, name="r")
            nc.scalar.activation(r, lnz, act.Exp, scale=-1.0)

            # ---- Emax tree (bf16), depth ~4 with 4 rotating slots ----
            emax = pp.tile([P, NF], bf16, name="emax")

            def vmax(a, b, nm):
                o = mxp.tile([P, NF], bf16, name=nm, tag="mx")
                nc.vector.tensor_tensor(o, a, b, op.max)
                return o

            m01 = vmax(E[0], E[1], "m01")
            m23 = vmax(E[2], E[3], "m23")
            h0 = vmax(m01, m23, "h0")      # frees 2 slots
            m45 = vmax(E[4], E[5], "m45")
            m67 = vmax(E[6], E[7], "m67")
            h1 = vmax(m45, m67, "h1")
            m89 = vmax(E[8], E[9], "m89")
            h2 = vmax(m89, E[10], "h2")
            h3 = vmax(h0, h1, "h3")
            nc.vector.tensor_tensor(emax, h3, h2, op.max)

            # ---- phase C: per-class products; pt/sumP/inter/PB via PE ----
            with (
                tc.tile_pool(name="ptpsum", bufs=1, space="PSUM") as ptp,
                tc.tile_pool(name="hpsum", bufs=1, space="PSUM") as hp,
            ):
                ptps = ptp.tile([P, NF], f32, name="ptps")
                spbank = hp.tile([P, W], f32, name="spbank")
                inbank = hp.tile([P, W], f32, name="inbank")
                for c in range(C):
                    # oh_c = [T == c] (bf16 0/1), fused count accum
                    oh = cp.tile([P, NF], bf16, name=f"oh{c}", tag="oh", bufs=2)
                    nc.vector.tensor_scalar(
                        oh, t16i, c, 0.0, op.is_equal, op.add,
                        accum_out=stats[:, COL_COUNT + c : COL_COUNT + c + 1],
                    )
                    # P_c = E_c * r (softmax probs, bf16)
                    pc = cp.tile([P, NF], bf16, name=f"pc{c}", tag="pc", bufs=2)
                    nc.vector.tensor_tensor(pc, E[c], r, op.mult)
                    # ohP_c = oh_c * P_c
                    ohp = cp.tile([P, NF], bf16, name=f"ohp{c}", tag="ohp", bufs=2)
                    nc.vector.tensor_tensor(ohp, oh, pc, op.mult)
                    for k in range(KB):
                        sl = slice(k * W, (k + 1) * W)
                        # sumP[c] += colsum(P_c) into spbank row c
                        nc.tensor.matmul(
                            spbank[:, :], ocols[c], pc[:, sl],
                            start=(c == 0 and k == 0),
                            stop=(c == C - 1 and k == KB - 1),
                        )
                        # inter[c] += colsum(ohP_c) into inbank row c
                        nc.tensor.matmul(
                            inbank[:, :], ocols[c], ohp[:, sl],
                            start=(c == 0 and k == 0),
                            stop=(c == C - 1 and k == KB - 1),
                        )
                        # pt += ohP_c (identity accumulate)
                        nc.tensor.matmul(
                            ptps[:, sl], identb, ohp[:, sl],
                            start=(c == 0),
                            stop=(c == C - 1),
                        )
                # per-class sums: partition-row c of the histo banks
                sp_sc = cp.tile([P, W], f32, name="sp_sc", tag="spsc", bufs=1)
                nc.vector.tensor_scalar(
                    sp_sc, spbank, 1, 0.0, op.mult, op.add,
                    accum_out=statsp[:, 0:1],
                )
                in_sc = cp.tile([P, W], f32, name="in_sc", tag="insc", bufs=1)
                nc.vector.tensor_scalar(
                    in_sc, inbank, 1, 0.0, op.mult, op.add,
                    accum_out=statsp[:, 1:2],
                )
                # pt plane (bf16) via ACT copy from PSUM
                pt = pp.tile([P, NF], bf16, name="pt")
                nc.scalar.copy(pt, ptps)

            # ---- PB: multi-hot pred bitmask via scaled-identity matmuls ----
            pb16 = pp.tile([P, NF], i16, name="pb16")
            with tc.tile_pool(name="pbpsum", bufs=1, space="PSUM") as pbp:
                pbps = pbp.tile([P, NF], f32, name="pbps")
                for c in range(1, C):
                    pm = cp.tile([P, NF], bf16, name=f"pm{c}", tag="pm", bufs=2)
                    nc.vector.tensor_tensor(pm, E[c], emax, op.is_ge)
                    for k in range(KB):
                        nc.tensor.matmul(
                            pbps[:, k * W : (k + 1) * W],
                            sids[c - 1],
                            pm[:, k * W : (k + 1) * W],
                            start=(c == 1),
                            stop=(c == C - 1),
                        )
                # PB -> int16 (values are exact small ints in fp32 PSUM)
                nc.vector.tensor_copy(pb16, pbps)


            # ---- focal ----
            lg = cp.tile([P, NF], bf16, name="lg", tag="lg", bufs=1)
            nc.scalar.activation(lg, pt, act.Ln)
            q = cp.tile([P, NF], bf16, name="q", tag="q", bufs=1)
            nc.vector.tensor_scalar(q, pt, -1.0, 1.0, op.mult, op.add)
            q2 = cp.tile([P, NF], bf16, name="q2", tag="q2", bufs=1)
            nc.scalar.square(q2, q)
            fsc = cp.tile([P, NF], bf16, name="fsc", tag="fsc", bufs=1)
            nc.vector.scalar_tensor_tensor(
                fsc, q2, 1.0, lg, op.mult, op.mult,
                accum_out=stats[:, COL_FOCAL : COL_FOCAL + 1],
            )




            _ccm2.__exit__(None, None, None)
            _mcm.__exit__(None, None, None)
            _ecm.__exit__(None, None, None)
            _lcm.__exit__(None, None, None)
            # ---- edge: bitmask planes (int16) ----
            _ccm = tc.tile_pool(name="convp", bufs=1)
            convp = _ccm.__enter__()
            NPAD = KB * (W + 2)  # padded [128, 4, 514]
            bmp = convp.tile([P, NPAD], i16, name="bmp")
            nc.gpsimd.memset(bmp, 0)
            bmp3 = bmp.rearrange("p (k w) -> p k w", w=W + 2)
            bm_c = bmp3[:, :, 1 : W + 1]     # center view [128,4,512]
            bm_l = bmp3[:, :, 0:W]
            bm_r = bmp3[:, :, 2 : W + 2]
            ones16 = convp.tile([P, NF], i16, name="ones16", tag="ones16", bufs=1)
            nc.gpsimd.memset(ones16, 1)
            t16i3 = t16i.rearrange("p (k w) -> p k w", w=W)
            nc.vector.tensor_tensor(bm_c, ones16.rearrange("p (k w) -> p k w", w=W),
                                    t16i3, op.logical_shift_left)

            def i16tile(nm, pool=None):
                return (pool or convp).tile([P, NF], i16, name=nm)

            v3 = lambda a: a.rearrange("p (k w) -> p k w", w=W)

            bmrow = i16tile("bmrow")   # l | r
            nc.vector.tensor_tensor(v3(bmrow), bm_l, bm_r, op.bitwise_or)
            bmrow3 = i16tile("bmrow3")  # l | c | r
            nc.vector.tensor_tensor(v3(bmrow3), v3(bmrow), bm_c, op.bitwise_or)
            bmar = i16tile("bmar")     # l & r
            nc.vector.tensor_tensor(v3(bmar), bm_l, bm_r, op.bitwise_and)
            bma3 = i16tile("bma3")     # l & c & r
            nc.vector.tensor_tensor(v3(bma3), v3(bmar), bm_c, op.bitwise_and)
            bmc_t = i16tile("bmc_t", pp)   # contiguous copy of center
            nc.vector.tensor_copy(v3(bmc_t), bm_c)

            # vertical shifts via SBUF->SBUF DMA (row +-1 with cross-block stitch)
            def vshift(src, nm_dn, nm_up):
                s3 = v3(src)
                dn = i16tile(nm_dn)
                up = i16tile(nm_up)
                d3, u3 = v3(dn), v3(up)
                nc.gpsimd.memset(dn, 0)
                nc.gpsimd.dma_start(d3[1:P, :, :], s3[0 : P - 1, :, :])
                nc.gpsimd.dma_start(d3[0:1, 1:KB, :], s3[P - 1 : P, 0 : KB - 1, :])
                nc.gpsimd.memset(up, 0)
                nc.gpsimd.dma_start(u3[0 : P - 1, :, :], s3[1:P, :, :])
                nc.gpsimd.dma_start(u3[P - 1 : P, 0 : KB - 1, :], s3[0:1, 1:KB, :])
                return dn, up

            odn, oup = vshift(bmrow3, "odn", "oup")
            adn, aup = vshift(bma3, "adn", "aup")
            bdn, bup = vshift(bmc_t, "bdn", "bup")

            bmor = i16tile("bmor", pp)
            nc.vector.tensor_tensor(bmor, odn, oup, op.bitwise_or)
            nc.vector.tensor_tensor(bmor, bmor, bmrow3, op.bitwise_or)
            bmand = i16tile("bmand", pp)
            nc.vector.tensor_tensor(bmand, adn, aup, op.bitwise_and)
            nc.vector.tensor_tensor(bmand, bmand, bma3, op.bitwise_and)
            bm4 = i16tile("bm4", pp)
            nc.vector.tensor_tensor(bm4, bdn, bup, op.bitwise_or)
            nc.vector.tensor_tensor(bm4, bm4, bmrow, op.bitwise_or)


            _ccm.__exit__(None, None, None)

            _ycm = tc.tile_pool(name="ypool", bufs=2)
            yp = _ycm.__enter__()

            # Y planes
            notpb = yp.tile([P, NF], i16, name="notpb", tag="ych", bufs=3)
            nc.vector.tensor_scalar(notpb, pb16, 0x7FF, None, op.bitwise_xor)
            notbm = yp.tile([P, NF], i16, name="notbm", tag="ych", bufs=3)
            nc.vector.tensor_scalar(notbm, bmc_t, 0x7FF, None, op.bitwise_xor)
            y1p = i16tile("y1p", pp)       # bm & ~PB (single-bit)
            nc.vector.tensor_tensor(y1p, bmc_t, notpb, op.bitwise_and)
            y2p = i16tile("y2p", pp)       # bmAND & ~PB (single-bit)
            nc.vector.tensor_tensor(y2p, bmand, notpb, op.bitwise_and)
            pbnb = yp.tile([P, NF], i16, name="pbnb", tag="ych", bufs=3)     # PB & ~bm
            nc.vector.tensor_tensor(pbnb, pb16, notbm, op.bitwise_and)
            y3p = i16tile("y3p", pp)       # PB & ~bm & bmOR (multi-bit)
            nc.vector.tensor_tensor(y3p, pbnb, bmor, op.bitwise_and)
            y4p = i16tile("y4p", pp)       # PB & ~bm & bm4 (multi-bit)
            nc.vector.tensor_tensor(y4p, pbnb, bm4, op.bitwise_and)


            # per-class accumulations
            def eq_acc(src, val, col, nm):
                # single-bit plane: count of [src == val], fused accum
                o = yp.tile([P, NF], i16, name=nm, tag="sbit", bufs=4)
                nc.vector.tensor_scalar(
                    o, src, val, 0.0, op.is_equal, op.add,
                    accum_out=stats[:, col : col + 1],
                )

            na = [0]

            def bit_acc(src, c, col, nm, acc_engine):
                # multi-bit plane: (src >> c) & 1, then arith accum
                b = yp.tile([P, NF], i16, name=nm, tag="sbit", bufs=4)
                nc.vector.tensor_scalar(
                    b, src, c, 1, op.logical_shift_right, op.bitwise_and
                )
                if acc_engine == "act":
                    oa = yp.tile([P, NF], f32, name=nm + "f", tag="bitf", bufs=1)
                    k = na[0]; na[0] += 1
                    nc.scalar.activation(
                        oa, b, act.Identity,
                        accum_out=statsa[:, k : k + 1],
                    )
                    acol_map.append((col, k))
                else:
                    o = yp.tile([P, NF], i16, name=nm + "a", tag="sbit", bufs=4)
                    nc.vector.tensor_scalar(
                        o, b, 1, 0.0, op.mult, op.add,
                        accum_out=stats[:, col : col + 1],
                    )

            acol_map = []
            for c in range(1, C):
                eq_acc(y1p, 1 << c, COL_Y1 + c - 1, f"by1_{c}")
                eq_acc(y2p, 1 << c, COL_Y2 + c - 1, f"by2_{c}")
                eq_acc(bmand, 1 << c, COL_DENN + c - 1, f"bdn{c}")
                bit_acc(y3p, c, COL_Y3 + c - 1, f"by3_{c}", "act")
                bit_acc(y4p, c, COL_Y4 + c - 1, f"by4_{c}",
                        "act" if c % 2 == 1 else "dve")
                bit_acc(bmor, c, COL_DENP + c - 1, f"bdp{c}",
                        "act" if c % 2 == 0 else "dve")

            nc.gpsimd.dma_start(stats_out[:, :], stats)
            nc.gpsimd.dma_start(statsa_out[:, :], statsa)
            nc.gpsimd.dma_start(statsp_out[:, :], statsp)
            _ycm.__exit__(None, None, None)

    nc.compile()
    return nc, acol_map


def _decode(stats_list):
    """stats_list: 8 arrays [128, NCOL] fp32 -> (total, dice, focal, edge)."""
    dices, focals, edges = [], [], []
    for s in stats_list:
        v = s.astype(np.float64).sum(axis=0)
        inter = v[COL_INTER : COL_INTER + 11]
        sump = v[COL_SUMP : COL_SUMP + 11]
        count = v[COL_COUNT : COL_COUNT + 11]
        dice = (2.0 * inter + EPS) / (sump + count + EPS)
        dices.append(dice.mean())
        focals.append(-0.25 * v[COL_FOCAL] / NPIX)
        ny1 = v[COL_Y1 : COL_Y1 + 10]
        ny2 = v[COL_Y2 : COL_Y2 + 10]
        ny3 = v[COL_Y3 : COL_Y3 + 10]
        ny4 = v[COL_Y4 : COL_Y4 + 10]
        denp = v[COL_DENP : COL_DENP + 10]
        denn = v[COL_DENN : COL_DENN + 10]
        num = (ny1 - ny2) + ES2 * ny3 + (E1 - ES2) * ny4
        den = denp - denn
        cls = np.where(den > 0, num / np.maximum(den, 1.0), 0.0)
        edges.append(cls.mean())
    dice_loss = 1.0 - float(np.mean(dices))
    focal_loss = float(np.mean(focals))
    edge_loss = float(np.mean(edges))
    total = dice_loss + focal_loss + edge_loss
    return (
        np.float32(total),
        np.float32(dice_loss),
        np.float32(focal_loss),
        np.float32(edge_loss),
    )


def kernel(inputs: np.ndarray, targets: np.ndarray):
    from concourse.bass_utils import run_bass_kernel_spmd

    if "nc" not in _cache:
        _cache["nc"], _cache["acol_map"] = _build()
    nc = _cache["nc"]

    inputs = np.ascontiguousarray(np.asarray(inputs, dtype=np.float32))
    targets = np.ascontiguousarray(np.asarray(targets, dtype=np.int32))
    in_maps = [{"x": inputs[b], "t": targets[b]} for b in range(B)]
    res = run_bass_kernel_spmd(nc, in_maps, core_ids=list(range(B)))
    _cache["last_result"] = res
    merged = []
    for rr in res.results:
        s_ = rr["stats"].astype(np.float64).copy()
        sa = rr["statsa"].astype(np.float64)
        for col, k in _cache["acol_map"]:
            s_[:, col] = sa[:, k]
        # sumP[c]/inter[c] live in partition-row c of statsp cols 0/1
        sp_ = rr["statsp"].astype(np.float64)
        s_[:, COL_SUMP : COL_SUMP + 11] = 0.0
        s_[0, COL_SUMP : COL_SUMP + 11] = sp_[0:11, 0]
        s_[:, COL_INTER : COL_INTER + 11] = 0.0
        s_[0, COL_INTER : COL_INTER + 11] = sp_[0:11, 1]
        merged.append(s_)
    return _decode(merged)



# revision 17
# speedup vs baseline: 1.1447x; 1.1447x over previous
"""CompoundLoss (dice + focal + edge) Trainium2 Bass kernel, v2.

Self-contained: hardcodes shapes [8,11,512,512] f32 logits + [8,512,512] i32
targets, shards batch across 8 NeuronCores (pure data parallel). Each core
reduces its image to per-partition fp32 accumulator columns; the host
finishes the tiny scalar math in fp64.

v2 layout/schedule notes:
  Row r = p*4 + k (p = partition, k = row-block): 8KB contiguous DMA lines,
  and vertical neighbors live in the free dim (k +- 1) except at partition
  boundaries, which are stitched into pad rows of a padded mask tile with
  two small SBUF->SBUF DMAs.
  All targets-derived edge-mask work + denN/denP counting overlaps the
  logit DMA window; the exp->Z->r chain runs concurrently (ACT/PE); the
  E-max tree runs on the Pool engine. Post-r: pm/PB first (PSUM bank
  sequencing), then dice/focal phase C, then bit-counts via single
  tensor_scalar ops (S-trick: sum(V>>c), host finishes S_c - 2*S_{c+1}).
"""

import numpy as np

B, C, H, W = 8, 11, 512, 512
P = 128
KB = H // P          # 4 row-blocks per partition
NF = KB * W          # 2048 free elems per partition
KP = KB + 2          # padded row-blocks
WP = W + 2           # padded row width
NPIX = H * W
EPS = 1e-6
E1 = float(np.exp(-1.0))
ES2 = float(np.exp(-np.sqrt(2.0)))

# stats column layout
COL_COUNT = 0          # 11
COL_FOCAL = 11         # 1
COL_A = 12             # 10 (c=1..10)  #(bm & PB == 1<<c)
COL_B = 22             # 10            #(bmand & PB == 1<<c)
COL_GS = 32            # 10            S_c of (bmor & PB)
COL_HS = 42            # 10            S_c of (bm4' & PB)
COL_DN = 52            # 10            #(bmand == 1<<c)
COL_DPS = 62           # 10            S_c of bmor
NCOL = 72

_cache = {}


def _build():
    import ml_dtypes
    import concourse.bacc as bacc
    import concourse.mybir as mybir
    from concourse.tile import TileContext

    f32 = mybir.dt.float32
    bf16 = mybir.dt.bfloat16
    i32 = mybir.dt.int32
    i16 = mybir.dt.int16
    op = mybir.AluOpType
    act = mybir.ActivationFunctionType

    nc = bacc.Bacc()
    x = nc.dram_tensor("x", [C, H, W], f32, kind="ExternalInput")
    t = nc.dram_tensor("t", [H, W], i32, kind="ExternalInput")
    stats_out = nc.dram_tensor("stats", [P, NCOL], f32, kind="ExternalOutput")
    statsp_out = nc.dram_tensor("statsp", [P, 4], f32, kind="ExternalOutput")

    # row r = p*KB + k
    xv = x[:, :, :].rearrange("c (p k) w -> c p k w", k=KB)
    tv = t[:, :].rearrange("(p k) w -> p k w", k=KB)

    ident_np = np.eye(P, dtype=np.float32)
    ident_d = nc.inline_tensor(ident_np.astype(ml_dtypes.bfloat16), name="ident")
    sid_np = np.stack([ident_np * float(1 << c) for c in range(1, C)])
    sid_d = nc.inline_tensor(sid_np.astype(ml_dtypes.bfloat16), name="sident")
    ocol_np = np.zeros((C, P, P), dtype=np.float32)
    for c_ in range(C):
        ocol_np[c_, :, c_] = 1.0
    ocol_d = nc.inline_tensor(ocol_np.astype(ml_dtypes.bfloat16), name="ocol")
    ones_d = nc.inline_tensor(np.ones((P, NF), dtype=np.int16), name="onesc")

    with TileContext(nc, pool_alloc_mode="queue") as tc:
        with (
            tc.tile_pool(name="persist", bufs=1) as pp,
            tc.tile_pool(name="scratch", bufs=2) as sp,
        ):
            # ---- constants on the ACT queue (keeps SP queue for bulk loads)
            identb = pp.tile([P, P], bf16, name="identb")
            nc.scalar.dma_start(identb, ident_d[:, :])
            ones16 = pp.tile([P, NF], i16, name="ones16")
            nc.scalar.dma_start(ones16, ones_d[:, :])
            sids = []
            for ci in range(C - 1):
                sid = pp.tile([P, P], bf16, name=f"sid{ci}", tag=f"sid{ci}")
                nc.scalar.dma_start(sid, sid_d[ci])
                sids.append(sid)
            ocols = []
            for ci in range(C):
                oc = pp.tile([P, P], bf16, name=f"oc{ci}", tag=f"oc{ci}")
                nc.scalar.dma_start(oc, ocol_d[ci])
                ocols.append(oc)

            stats = pp.tile([P, NCOL], f32, name="stats")
            statsp = pp.tile([P, 4], f32, name="statsp")

            # ---- targets first on the SP queue
            ts32 = sp.tile([P, NF], i32, name="ts32", tag="ts32", bufs=1)
            nc.sync.dma_start(ts32.rearrange("p (k w) -> p k w", w=W), tv)
            t16i = pp.tile([P, NF], i16, name="t16i")
            nc.vector.tensor_copy(t16i, ts32)
            t16i3 = t16i.rearrange("p (k w) -> p k w", w=W)

            # ---- logits load + exp + Z(PSUM) + Pool max-tree ----
            E = []
            _ecm = tc.tile_pool(name="epool", bufs=11)
            _mcm = tc.tile_pool(name="mxpool", bufs=5)
            _lcm = tc.tile_pool(name="lpool", bufs=3)
            epool = _ecm.__enter__()
            mxp = _mcm.__enter__()
            lpool = _lcm.__enter__()

            emax = pp.tile([P, NF], bf16, name="emax")
            tree = {}

            def pmax(a, b, nm, out=None):
                o = out if out is not None else mxp.tile(
                    [P, NF], bf16, name=nm, tag="mx"
                )
                nc.vector.tensor_tensor(o, a, b, op.max)
                return o

            def tree_step(c):
                # pairwise maxes as planes land; post-E10 depth = 1
                if c == 1:
                    tree["m01"] = pmax(E[0], E[1], "m01")
                elif c == 3:
                    tree["m23"] = pmax(E[2], E[3], "m23")
                    tree["A"] = pmax(tree["m01"], tree["m23"], "tA")
                elif c == 5:
                    tree["m45"] = pmax(E[4], E[5], "m45")
                elif c == 7:
                    tree["m67"] = pmax(E[6], E[7], "m67")
                    tree["B"] = pmax(tree["m45"], tree["m67"], "tB")
                    tree["C2"] = pmax(tree["A"], tree["B"], "tC")
                elif c == 9:
                    tree["m89"] = pmax(E[8], E[9], "m89")
                    tree["D"] = pmax(tree["C2"], tree["m89"], "tD")
                elif c == 10:
                    pmax(tree["D"], E[10], "emax", out=emax)

            _zcm = tc.tile_pool(name="zpsum", bufs=1, space="PSUM")
            zp = _zcm.__enter__()
            zps = zp.tile([P, NF], f32, name="zps")
            for c in range(C):
                lb = lpool.tile([P, NF], f32, name=f"lb{c}", tag="lb")
                nc.sync.dma_start(lb.rearrange("p (k w) -> p k w", w=W), xv[c])
                e = epool.tile([P, NF], bf16, name=f"e{c}", tag="e")
                nc.scalar.activation(e, lb, act.Exp)
                E.append(e)
                for k in range(KB):
                    nc.tensor.matmul(
                        zps[:, k * W : (k + 1) * W],
                        identb,
                        e[:, k * W : (k + 1) * W],
                        start=(c == 0),
                        stop=(c == C - 1),
                    )
                tree_step(c)

            # ---- r = 1/Z via exp(-ln Z) ----
            lnz = sp.tile([P, NF], f32, name="lnz", tag="lnz", bufs=1)
            nc.scalar.activation(lnz, zps, act.Ln)
            _zcm.__exit__(None, None, None)
            _lcm.__exit__(None, None, None)
            _mcm.__exit__(None, None, None)
            r = pp.tile([P, NF], bf16, name="r")
            nc.scalar.activation(r, lnz, act.Exp, scale=-1.0)

            # ---- edge masks from targets (overlaps the load window) ----
            _bcm = tc.tile_pool(name="bpool", bufs=1)
            bp = _bcm.__enter__()
            bmp = pp.tile([P, KP * WP], i16, name="bmp")   # padded [6, 514]
            nc.gpsimd.memset(bmp, 0)
            bmp3 = bmp.rearrange("p (k w) -> p k w", w=WP)
            bm_c = bmp3[:, 1 : KB + 1, 1 : W + 1]          # real rows/cols
            nc.vector.tensor_tensor(bm_c, ones16.rearrange("p (k w) -> p k w", w=W),
                                    t16i3, op.logical_shift_left)
            # stitch pad rows from neighbor partitions (image top/bottom stay 0)
            nc.scalar.dma_start(bmp3[1:P, 0:1, 1 : W + 1],
                                bmp3[0 : P - 1, KB : KB + 1, 1 : W + 1])
            nc.scalar.dma_start(bmp3[0 : P - 1, KB + 1 : KB + 2, 1 : W + 1],
                                bmp3[1:P, 1:2, 1 : W + 1])

            bm_l = bmp3[:, :, 0:W]
            bm_ctr = bmp3[:, :, 1 : W + 1]
            bm_r = bmp3[:, :, 2 : W + 2]

            def rowtile(nm):
                return bp.tile([P, KP * W], i16, name=nm)

            v6 = lambda a: a.rearrange("p (k w) -> p k w", w=W)
            bmrow = rowtile("bmrow")       # l | r, all 6 row-blocks
            nc.vector.tensor_tensor(v6(bmrow), bm_l, bm_r, op.bitwise_or)
            bmrow3 = rowtile("bmrow3")     # l | c | r
            nc.vector.tensor_tensor(v6(bmrow3), v6(bmrow), bm_ctr, op.bitwise_or)
            bmar = rowtile("bmar")         # l & r
            nc.vector.tensor_tensor(v6(bmar), bm_l, bm_r, op.bitwise_and)
            bma3 = rowtile("bma3")         # l & c & r
            nc.vector.tensor_tensor(v6(bma3), v6(bmar), bm_ctr, op.bitwise_and)

            def mid(tile_):                # real rows 1..4 of a 6-row tile
                return v6(tile_)[:, 1 : KB + 1, :]

            def up(tile_):
                return v6(tile_)[:, 0:KB, :]

            def dn(tile_):
                return v6(tile_)[:, 2 : KB + 2, :]

            bmor = pp.tile([P, NF], i16, name="bmor")
            bmor3 = bmor.rearrange("p (k w) -> p k w", w=W)
            nc.vector.tensor_tensor(bmor3, up(bmrow3), dn(bmrow3), op.bitwise_or)
            nc.vector.tensor_tensor(bmor3, bmor3, mid(bmrow3), op.bitwise_or)
            bmand = pp.tile([P, NF], i16, name="bmand")
            bmand3 = bmand.rearrange("p (k w) -> p k w", w=W)
            nc.vector.tensor_tensor(bmand3, up(bma3), dn(bma3), op.bitwise_and)
            nc.vector.tensor_tensor(bmand3, bmand3, mid(bma3), op.bitwise_and)
            # bm4' = up | dn | l | r | center
            bm4p = pp.tile([P, NF], i16, name="bm4p")
            bm4p3 = bm4p.rearrange("p (k w) -> p k w", w=W)
            nc.vector.tensor_tensor(
                bm4p3, bmp3[:, 0:KB, 1 : W + 1], bmp3[:, 2 : KB + 2, 1 : W + 1],
                op.bitwise_or)
            nc.vector.tensor_tensor(bm4p3, bm4p3, mid(bmrow), op.bitwise_or)
            nc.vector.tensor_tensor(bm4p3, bm4p3, bm_c, op.bitwise_or)
            _bcm.__exit__(None, None, None)

            # ---- in-window counts (single tensor_scalar each) ----
            _ycm = tc.tile_pool(name="ypool", bufs=2)
            yp = _ycm.__enter__()

            def eq_acc(src, val, col, nm, eng=nc.vector):
                o = yp.tile([P, NF], i16, name=nm, tag="cnt", bufs=2)
                eng.tensor_scalar(
                    o, src, val, 0.0, op.is_equal, op.add,
                    accum_out=stats[:, col : col + 1],
                )

            def s_acc(src, c, col, nm, eng=nc.vector):
                # direct bit-c count: (src >> c) & 1, then arith sum
                o = yp.tile([P, NF], i16, name=nm, tag="cnt", bufs=2)
                nc.vector.tensor_scalar(
                    o, src, c, 1, op.logical_shift_right, op.bitwise_and
                )
                o2 = yp.tile([P, NF], i16, name=nm + "s", tag="cnt", bufs=2)
                nc.vector.tensor_scalar(
                    o2, o, 1, 0.0, op.mult, op.add,
                    accum_out=stats[:, col : col + 1],
                )

            for c in range(1, C):
                eq_acc(bmand, 1 << c, COL_DN + c - 1, f"dn{c}")
                s_acc(bmor, c, COL_DPS + c - 1, f"dp{c}")

            # ---- pm / PB (PSUM banks freed by Z) ----
            pb16 = pp.tile([P, NF], i16, name="pb16")
            _pcm = tc.tile_pool(name="pbpsum", bufs=1, space="PSUM")
            pbp = _pcm.__enter__()
            pbps = pbp.tile([P, NF], f32, name="pbps")
            for c in range(1, C):
                pm = sp.tile([P, NF], bf16, name=f"pm{c}", tag="pm", bufs=2)
                nc.vector.tensor_tensor(pm, E[c], emax, op.is_ge)
                for k in range(KB):
                    nc.tensor.matmul(
                        pbps[:, k * W : (k + 1) * W],
                        sids[c - 1],
                        pm[:, k * W : (k + 1) * W],
                        start=(c == 1),
                        stop=(c == C - 1),
                    )
            nc.vector.tensor_copy(pb16, pbps)
            _pcm.__exit__(None, None, None)

            # ---- dice/focal phase C ----
            _ccm = tc.tile_pool(name="cpool", bufs=2)
            cp = _ccm.__enter__()
            _ptm = tc.tile_pool(name="ptpsum", bufs=1, space="PSUM")
            _hpm = tc.tile_pool(name="hpsum", bufs=1, space="PSUM")
            ptp = _ptm.__enter__()
            hp = _hpm.__enter__()
            ptps = ptp.tile([P, NF], f32, name="ptps")
            spbank = hp.tile([P, W], f32, name="spbank")
            inbank = hp.tile([P, W], f32, name="inbank")
            for c in range(C):
                oh = cp.tile([P, NF], bf16, name=f"oh{c}", tag="oh", bufs=2)
                nc.vector.tensor_scalar(
                    oh, t16i, c, 0.0, op.is_equal, op.add,
                    accum_out=stats[:, COL_COUNT + c : COL_COUNT + c + 1],
                )
                pc = cp.tile([P, NF], bf16, name=f"pc{c}", tag="pc", bufs=2)
                nc.vector.tensor_tensor(pc, E[c], r, op.mult)
                ohp = cp.tile([P, NF], bf16, name=f"ohp{c}", tag="ohp", bufs=2)
                nc.vector.tensor_tensor(ohp, oh, pc, op.mult)
                for k in range(KB):
                    sl = slice(k * W, (k + 1) * W)
                    nc.tensor.matmul(
                        spbank[:, :], ocols[c], pc[:, sl],
                        start=(c == 0 and k == 0),
                        stop=(c == C - 1 and k == KB - 1),
                    )
                    nc.tensor.matmul(
                        inbank[:, :], ocols[c], ohp[:, sl],
                        start=(c == 0 and k == 0),
                        stop=(c == C - 1 and k == KB - 1),
                    )
                    nc.tensor.matmul(
                        ptps[:, sl], identb, ohp[:, sl],
                        start=(c == 0),
                        stop=(c == C - 1),
                    )
            sp_sc = cp.tile([P, W], f32, name="sp_sc", tag="spsc", bufs=1)
            nc.vector.tensor_scalar(
                sp_sc, spbank, 1, 0.0, op.mult, op.add,
                accum_out=statsp[:, 0:1],
            )
            in_sc = cp.tile([P, W], f32, name="in_sc", tag="insc", bufs=1)
            nc.vector.tensor_scalar(
                in_sc, inbank, 1, 0.0, op.mult, op.add,
                accum_out=statsp[:, 1:2],
            )
            _hpm.__exit__(None, None, None)
            pt = pp.tile([P, NF], bf16, name="pt")
            nc.scalar.copy(pt, ptps)
            _ptm.__exit__(None, None, None)

            # ---- products vs PB (Pool) + post counts ----
            aPB = pp.tile([P, NF], i16, name="aPB")
            nc.vector.tensor_tensor(aPB, bm_c, pb16.rearrange(
                "p (k w) -> p k w", w=W), op.bitwise_and)
            bPB = pp.tile([P, NF], i16, name="bPB")
            nc.vector.tensor_tensor(bPB, bmand, pb16, op.bitwise_and)
            gPB = pp.tile([P, NF], i16, name="gPB")
            nc.vector.tensor_tensor(gPB, bmor, pb16, op.bitwise_and)
            hPB = pp.tile([P, NF], i16, name="hPB")
            nc.vector.tensor_tensor(hPB, bm4p, pb16, op.bitwise_and)

            # ---- focal ----
            lg = cp.tile([P, NF], bf16, name="lg", tag="lg", bufs=1)
            nc.scalar.activation(lg, pt, act.Ln)
            q = cp.tile([P, NF], bf16, name="q", tag="q", bufs=1)
            nc.vector.tensor_scalar(q, pt, -1.0, 1.0, op.mult, op.add)
            q2 = cp.tile([P, NF], bf16, name="q2", tag="q2", bufs=1)
            nc.scalar.square(q2, q)
            q2lg = cp.tile([P, NF], bf16, name="q2lg", tag="q2lg", bufs=1)
            nc.vector.tensor_tensor(q2lg, q2, lg, op.mult)
            fsc = cp.tile([P, NF], bf16, name="fsc", tag="fsc", bufs=1)
            nc.vector.tensor_scalar(
                fsc, q2lg, 1.0, 0.0, op.mult, op.add,
                accum_out=stats[:, COL_FOCAL : COL_FOCAL + 1],
            )

            for c in range(1, C):
                eq_acc(aPB, 1 << c, COL_A + c - 1, f"a{c}")
                eq_acc(bPB, 1 << c, COL_B + c - 1, f"b{c}")
                s_acc(gPB, c, COL_GS + c - 1, f"g{c}")
                s_acc(hPB, c, COL_HS + c - 1, f"h{c}")

            nc.sync.dma_start(stats_out[:, :], stats)
            nc.sync.dma_start(statsp_out[:, :], statsp)

            for cm in [_ccm, _ycm, _ecm]:
                cm.__exit__(None, None, None)

    nc.compile()
    return nc


def _decode(stats_list, statsp_list):
    dices, focals, edges = [], [], []
    for s_, sp_ in zip(stats_list, statsp_list):
        v = s_.astype(np.float64).sum(axis=0)
        count = v[COL_COUNT : COL_COUNT + 11]
        sumP = sp_.astype(np.float64)[0:11, 0]
        inter = sp_.astype(np.float64)[0:11, 1]
        dice = (2.0 * inter + EPS) / (sumP + count + EPS)
        dices.append(dice.mean())
        focals.append(-0.25 * v[COL_FOCAL] / NPIX)

        A = v[COL_A : COL_A + 10]
        Bv = v[COL_B : COL_B + 10]
        G = v[COL_GS : COL_GS + 10]
        Hc = v[COL_HS : COL_HS + 10]
        Dn = v[COL_DN : COL_DN + 10]
        Dp = v[COL_DPS : COL_DPS + 10]
        y1 = count[1:11] - A
        y2 = Dn - Bv
        y3 = G - A
        y4 = Hc - A
        num = (y1 - y2) + ES2 * y3 + (E1 - ES2) * y4
        den = Dp - Dn
        cls = np.where(den > 0, num / np.maximum(den, 1.0), 0.0)
        edges.append(cls.mean())
    dice_loss = 1.0 - float(np.mean(dices))
    focal_loss = float(np.mean(focals))
    edge_loss = float(np.mean(edges))
    total = dice_loss + focal_loss + edge_loss
    return (
        np.float32(total),
        np.float32(dice_loss),
        np.float32(focal_loss),
        np.float32(edge_loss),
    )


def kernel(inputs: np.ndarray, targets: np.ndarray):
    from concourse.bass_utils import run_bass_kernel_spmd

    if "nc" not in _cache:
        _cache["nc"] = _build()
    nc = _cache["nc"]

    inputs = np.ascontiguousarray(np.asarray(inputs, dtype=np.float32))
    targets = np.ascontiguousarray(np.asarray(targets, dtype=np.int32))
    in_maps = [{"x": inputs[b], "t": targets[b]} for b in range(B)]
    res = run_bass_kernel_spmd(nc, in_maps, core_ids=list(range(B)))
    _cache["last_result"] = res
    return _decode(
        [rr["stats"] for rr in res.results],
        [rr["statsp"] for rr in res.results],
    )
